# revision 15
# baseline (speedup 1.0000x reference)
"""Cross-attention Bass kernel for Trainium2.

Problem (per batch, data-parallel over 8 batches -> 8 NeuronCores):
    q = query @ W_q          [2048, 64]
    k = key   @ W_k          [2048, 64]
    v = key   @ W_v          [2048, 64]
    scores = q @ k.T         [2048, 2048]
    attn = softmax(scores, axis=-1)
    out = attn @ v           [2048, 64]

Strategy (per core), cost-model driven:
  * All big matmuls run at 1 cycle/row: scores in fp32r (moving free dim
    512 >= 256), attn@v in bf16.  The fp32 baseline paid 4 cycles/row.
  * W_q is folded into the K side: Gt = W_k @ W_q^T (one 128x128 matmul),
    N_t = (Gt^T) @ keyT_tile = W_q W_k^T keyT tile.  scoresT_t = N_t^T @
    queryT.  This removes separate q/k projections from the critical path.
  * attn@v is computed with l_q on PSUM partitions: out[q, 65] accumulated
    over the 16 k-tiles with lhsT = exp-scores block (stationary), rhs =
    [v_t | ones].  Free dim is 65 instead of 1024 -> half the PE cost of
    the baseline orientation, the softmax denominator rides along as
    column 64, and the output needs no epilogue transpose.
  * ACT (ScalarE) does nothing but the 32 exp instructions [128,1024]
    PSUM->SBUF(bf16); that is the critical path (~33 us).  Pool (gpsimd)
    does all PSUM->SBUF staging copies, DVE does the epilogue
    reciprocal+scale, SP/DVE/ACT sequencers issue the DMA rings.
  * PE p-state: junk transposes at t~0.3us keep the PE busy so the clock
    ramps to 2.4 GHz before the real work starts.
  * PSUM budget (8 banks): scores 2x[128,1024] (4) + attn@v accumulators
    [128,1024] (2, 8 q-tiles packed at 128-col offsets) + staging 2x1 (2).
  * k-tile staging (transpose -> N_t -> vP_t) is streamed just-in-time
    into chunk 0 of the main loop so the first exp starts at ~4.5 us.
"""

import numpy as np

import concourse.bass as bass
import concourse.bacc as bacc
import concourse.mybir as mybir
import concourse.tile as tile
from concourse import bass_utils
from concourse.masks import make_identity

F32 = mybir.dt.float32
F32R = mybir.dt.float32r
BF16 = mybir.dt.bfloat16
AF = mybir.ActivationFunctionType

B = 8
L = 2048
D = 128
E = 64
NT = L // 128           # 16 k-tiles (and q-tiles)
CHUNK = 1024            # l_q chunk
NCHUNK = L // CHUNK     # 2
NQT = CHUNK // 128      # 8 q-tiles per chunk
NWARM = 10              # junk transposes to ramp the PE clock
DEBUG_DUMPS = False


def _build(nc: bass.Bass, tc: tile.TileContext, out, query, key, wq_d, wk_d, wv_d, ctx,
           dbg_out=None):
    # ---------------- constants ----------------
    const = ctx.enter_context(tc.tile_pool(name="const", bufs=1))
    ident = const.tile([128, 128], F32)
    make_identity(nc, ident[:])

    # Warm the ACT exp table early (pulls the ~1.3us table load into the
    # DMA-wait window).
    warm = const.tile([128, 1], F32)
    nc.gpsimd.memset(warm[:], 0.0)
    nc.scalar.activation(warm[:], warm[:], AF.Exp)

    wq = const.tile([128, E], F32)
    wk = const.tile([128, E], F32)
    wv = const.tile([128, E], F32)

    qn = const.tile([128, L], F32)      # query natural, tile j at cols 128j
    kn = const.tile([128, L], F32)
    qTd = const.tile([128, L], F32R)    # queryT [d, l] (fp32r for scores)
    kTd = const.tile([128, L], F32)     # keyT   [d, l]
    wqt = const.tile([64, 128], F32)    # W_q^T [e, d]
    wkt = const.tile([64, 128], F32)
    gt = const.tile([128, 128], F32)    # G^T = W_k W_q^T
    nsb = const.tile([128, L], F32R)    # N_t = G @ keyT, tile t at cols 128t
    vag = const.tile([128, 65 * NT], BF16)  # [v_t | ones] per k-tile
    nc.gpsimd.memset(vag[:], 1.0)

    # ---------------- DMA issue ----------------
    q4 = query.rearrange("(c t p) d -> c p t d", t=4, p=128)  # [4,128,4,128]
    k4 = key.rearrange("(c t p) d -> c p t d", t=4, p=128)
    # ACT ring (idle until the exp era): weights then key halves 0,1.
    # SP ring: query halves then key halves 2,3.
    nc.scalar.dma_start(wq[:], wq_d[:])
    nc.scalar.dma_start(wk[:], wk_d[:])
    nc.scalar.dma_start(wv[:], wv_d[:])
    for j in range(2):
        nc.sync.dma_start(
            qn[:, 512 * j:512 * (j + 1)].rearrange("p (t d) -> p t d", d=128), q4[j])
        nc.scalar.dma_start(
            kn[:, 512 * j:512 * (j + 1)].rearrange("p (t d) -> p t d", d=128), k4[j])
    for j in range(2, 4):
        nc.sync.dma_start(
            kn[:, 512 * j:512 * (j + 1)].rearrange("p (t d) -> p t d", d=128), k4[j])
    for j in range(2, 4):
        nc.sync.dma_start(
            qn[:, 512 * j:512 * (j + 1)].rearrange("p (t d) -> p t d", d=128), q4[j])

    # ---------------- PSUM pools (whole kernel) ----------------
    st_pool = ctx.enter_context(tc.tile_pool(name="st", bufs=2, space="PSUM"))
    sc_pool = ctx.enter_context(tc.tile_pool(name="sc", bufs=2, space="PSUM"))
    ac_pool = ctx.enter_context(tc.tile_pool(name="ac", bufs=1, space="PSUM"))
    ex_pool = ctx.enter_context(tc.tile_pool(name="ex", bufs=3))
    ob_pool = ctx.enter_context(tc.tile_pool(name="ob", bufs=2))
    rc_pool = ctx.enter_context(tc.tile_pool(name="rc", bufs=2))

    # PE p-state warm-up: junk transposes with no readers.
    for i in range(NWARM):
        junk = st_pool.tile([128, 128], F32, tag="st", name=f"junk{i}")
        nc.tensor.transpose(junk[:], ident[:], ident[:])

    def copy(eng, dst, src):
        # GPSIMD cannot read PSUM; staging copies go on DVE (or ACT when the
        # exp era hasn't started yet).
        if eng is nc.scalar:
            nc.scalar.copy(dst, src)
        else:
            eng.tensor_copy(dst, src)

    def trans(dst, src_ap, eng=None):
        """PE transpose of [p,f] sbuf block -> [f,p] psum -> copy to dst."""
        np_, nf = src_ap.shape
        p = st_pool.tile([128, 128], F32, tag="st", name="tp")
        nc.tensor.transpose(p[0:nf, 0:np_], src_ap, ident[0:np_, 0:np_])
        copy(eng or nc.vector, dst, p[0:nf, 0:np_])

    def stage_ktile(s):
        """keyT tile s, N_s, vP_s."""
        trans(kTd[:, 128 * s:128 * (s + 1)], kn[:, 128 * s:128 * (s + 1)])
        p = st_pool.tile([128, 128], F32, tag="st", name="np")
        nc.tensor.matmul(p[:], gt[:], kTd[:, 128 * s:128 * (s + 1)],
                         start=True, stop=True)
        nc.vector.tensor_copy(nsb[:, 128 * s:128 * (s + 1)], p[:])
        pv = st_pool.tile([128, E], F32, tag="st", name="vp")
        nc.tensor.matmul(pv[:], kTd[:, 128 * s:128 * (s + 1)], wv[:],
                         start=True, stop=True)
        nc.vector.tensor_copy(vag[:, 65 * s:65 * s + 64], pv[:])

    def stage_qtile(j, eng=None):
        trans(qTd[:, 128 * j:128 * (j + 1)], qn[:, 128 * j:128 * (j + 1)], eng)

    def emit_scores(c, t):
        ps = sc_pool.tile([128, CHUNK], F32, tag="sc", name="ps")
        for j in range(CHUNK // 512):
            qs = slice(CHUNK * c + 512 * j, CHUNK * c + 512 * (j + 1))
            nc.tensor.matmul(
                ps[:, 512 * j:512 * (j + 1)],
                nsb[:, 128 * t:128 * (t + 1)],
                qTd[:, qs],
                start=True, stop=True,
            )
        return ps

    # ---------------- weight prep + phase-1 staging ----------------
    trans(wqt[:], wq[:])
    trans(wkt[:], wk[:])
    pg = st_pool.tile([128, 128], F32, tag="st", name="pg")
    nc.tensor.matmul(pg[:], wkt[:], wqt[:], start=True, stop=True)
    nc.vector.tensor_copy(gt[:], pg[:])

    JIT = False
    if JIT:
        stage_ktile(0)
        for j in range(4):
            stage_qtile(j)
        stage_ktile(1)
        for j in range(4, 8):
            stage_qtile(j)
    else:
        for s in range(NT):
            stage_ktile(s)
        for j in range(NT):
            stage_qtile(j)

    ps = emit_scores(0, 0)

    # ---------------- main loop ----------------
    dbg = ctx.enter_context(tc.tile_pool(name="dbg", bufs=1)) if DEBUG_DUMPS else None
    acc = None
    for c in range(NCHUNK):
        acc = ac_pool.tile([128, CHUNK], F32, tag="ac", name="acc")
        for t in range(NT):
            ex = ex_pool.tile([128, CHUNK], BF16, tag="ex", name="ex")
            nc.scalar.activation(ex[:], ps[:], AF.Exp)
            if DEBUG_DUMPS and (c, t) == (1, 0):
                d_sc = dbg.tile([128, CHUNK], F32, name="d_sc")
                nc.vector.tensor_copy(d_sc[:], ps[:])
                nc.sync.dma_start(dbg_out["sc10"], d_sc[:])
                d_ex = dbg.tile([128, CHUNK], F32, name="d_ex")
                nc.vector.tensor_copy(d_ex[:], ex[:])
                nc.sync.dma_start(dbg_out["ex10"], d_ex[:])

            # PE work independent of ex_t: JIT staging + next scores tile.
            if c == 0 and JIT:
                s = t + 2
                if s < NT:
                    stage_ktile(s)
                if 10 <= t <= 13:
                    stage_qtile(2 * (t - 10) + 8)
                    stage_qtile(2 * (t - 10) + 9)
            nc_, nt_ = (c, t + 1) if t + 1 < NT else (c + 1, 0)
            if nc_ < NCHUNK:
                ps = emit_scores(nc_, nt_)

            # attn@v for tile t (waits on ex_t).  PSUM start=True zeroes the
            # WHOLE bank, so only the first group per bank (j=0 -> bank 0,
            # j=4 -> bank 1) starts; the other groups accumulate onto the
            # zeroed bank with start=False (PE executes in emission order).
            for j in range(NQT):
                nc.tensor.matmul(
                    acc[:, 128 * j:128 * j + 65],
                    ex[:, 128 * j:128 * (j + 1)],
                    vag[:, 65 * t:65 * t + 65],
                    start=(t == 0 and j % 4 == 0), stop=(t == NT - 1),
                    skip_group_check=True,
                )

        if DEBUG_DUMPS:
            d_ac = dbg.tile([128, CHUNK], F32, name="d_ac")
            nc.vector.tensor_copy(d_ac[:], acc[:])
            nc.sync.dma_start(dbg_out[f"acc{c}"], d_ac[:])
        # epilogue for chunk c
        osb = ob_pool.tile([128, 64 * NQT], F32, tag="ob", name="osb")
        for j in range(NQT):
            rec = rc_pool.tile([128, 1], F32, tag="rc", name="rec")
            nc.vector.reciprocal(rec[:], acc[:, 128 * j + 64:128 * j + 65])
            nc.vector.tensor_scalar_mul(
                osb[:, 64 * j:64 * (j + 1)], acc[:, 128 * j:128 * j + 64], rec[:])
        o8 = out.rearrange("(c j p) e -> c p j e", p=128, j=NQT)  # [2,128,8,64]
        nc.sync.dma_start(o8[c], osb[:].rearrange("p (j e) -> p j e", e=64))


def build_nc() -> bass.Bass:
    nc = bacc.Bacc("TRN2", target_bir_lowering=False, debug=False,
                   enable_asserts=False, num_devices=B)
    query = nc.dram_tensor("query", [L, D], F32, kind="ExternalInput").ap()
    key = nc.dram_tensor("key", [L, D], F32, kind="ExternalInput").ap()
    wq = nc.dram_tensor("W_q", [D, E], F32, kind="ExternalInput").ap()
    wk = nc.dram_tensor("W_k", [D, E], F32, kind="ExternalInput").ap()
    wv = nc.dram_tensor("W_v", [D, E], F32, kind="ExternalInput").ap()
    out = nc.dram_tensor("out", [L, E], F32, kind="ExternalOutput").ap()
    dbg_out = None
    if DEBUG_DUMPS:
        dbg_out = {
            n: nc.dram_tensor(n, [128, CHUNK], F32, kind="ExternalOutput").ap()
            for n in ("sc10", "ex10", "acc0", "acc1")
        }
    from contextlib import ExitStack
    with tile.TileContext(nc) as tc:
        with ExitStack() as ctx:
            _build(nc, tc, out, query, key, wq, wk, wv, ctx, dbg_out)
    nc.compile()
    return nc


_NC_CACHE = None


def kernel(**inputs) -> np.ndarray:
    global _NC_CACHE
    if _NC_CACHE is None:
        _NC_CACHE = build_nc()
    nc = _NC_CACHE
    q = np.ascontiguousarray(np.asarray(inputs["query"], dtype=np.float32))
    k = np.ascontiguousarray(np.asarray(inputs["key"], dtype=np.float32))
    wq = np.ascontiguousarray(np.asarray(inputs["W_q"], dtype=np.float32))
    wk = np.ascontiguousarray(np.asarray(inputs["W_k"], dtype=np.float32))
    wv = np.ascontiguousarray(np.asarray(inputs["W_v"], dtype=np.float32))
    in_maps = [
        {"query": q[b], "key": k[b], "W_q": wq, "W_k": wk, "W_v": wv}
        for b in range(B)
    ]
    res = bass_utils.run_bass_kernel_spmd(nc, in_maps, core_ids=list(range(B)))
    return np.stack([r["out"] for r in res.results], axis=0)


# revision 16
# speedup vs baseline: 1.2139x; 1.2139x over previous
"""Cross-attention Bass kernel for Trainium2.

Problem (per batch, data-parallel over 8 batches -> 8 NeuronCores):
    q = query @ W_q          [2048, 64]
    k = key   @ W_k          [2048, 64]
    v = key   @ W_v          [2048, 64]
    scores = q @ k.T         [2048, 2048]
    attn = softmax(scores, axis=-1)
    out = attn @ v           [2048, 64]

Strategy (per core), cost-model driven:
  * All big matmuls run at 1 cycle/row: scores in fp32r (moving free dim
    512 >= 256), attn@v in bf16.  The fp32 baseline paid 4 cycles/row.
  * W_q is folded into the K side: Gt = W_k @ W_q^T (one 128x128 matmul),
    N_t = (Gt^T) @ keyT_tile = W_q W_k^T keyT tile.  scoresT_t = N_t^T @
    queryT.  This removes separate q/k projections from the critical path.
  * attn@v is computed with l_q on PSUM partitions: out[q, 65] accumulated
    over the 16 k-tiles with lhsT = exp-scores block (stationary), rhs =
    [v_t | ones].  Free dim is 65 instead of 1024 -> half the PE cost of
    the baseline orientation, the softmax denominator rides along as
    column 64, and the output needs no epilogue transpose.
  * ACT (ScalarE) does nothing but the 32 exp instructions [128,1024]
    PSUM->SBUF(bf16); that is the critical path (~33 us).  Pool (gpsimd)
    does all PSUM->SBUF staging copies, DVE does the epilogue
    reciprocal+scale, SP/DVE/ACT sequencers issue the DMA rings.
  * PE p-state: junk transposes at t~0.3us keep the PE busy so the clock
    ramps to 2.4 GHz before the real work starts.
  * PSUM budget (8 banks): scores 2x[128,1024] (4) + attn@v accumulators
    [128,1024] (2, 8 q-tiles packed at 128-col offsets) + staging 2x1 (2).
  * k-tile staging (transpose -> N_t -> vP_t) is streamed just-in-time
    into chunk 0 of the main loop so the first exp starts at ~4.5 us.
"""

import numpy as np

import concourse.bass as bass
import concourse.bacc as bacc
import concourse.mybir as mybir
import concourse.tile as tile
from concourse import bass_utils
from concourse.masks import make_identity

F32 = mybir.dt.float32
F32R = mybir.dt.float32r
BF16 = mybir.dt.bfloat16
AF = mybir.ActivationFunctionType

B = 8
L = 2048
D = 128
E = 64
NT = L // 128           # 16 k-tiles (and q-tiles)
CHUNK = 1024            # l_q chunk
NCHUNK = L // CHUNK     # 2
NQT = CHUNK // 128      # 8 q-tiles per chunk
NWARM = 10              # junk transposes to ramp the PE clock
DEBUG_DUMPS = False


def _build(nc: bass.Bass, tc: tile.TileContext, out, query, key, wq_d, wk_d, wv_d, ctx,
           dbg_out=None):
    # ---------------- constants ----------------
    const = ctx.enter_context(tc.tile_pool(name="const", bufs=1))
    ident = const.tile([128, 128], F32)
    make_identity(nc, ident[:])

    # Warm the ACT exp table early (pulls the ~1.3us table load into the
    # DMA-wait window).
    warm = const.tile([128, 1], F32)
    nc.gpsimd.memset(warm[:], 0.0)
    nc.scalar.activation(warm[:], warm[:], AF.Exp)

    wq = const.tile([128, E], F32)
    wk = const.tile([128, E], F32)
    wv = const.tile([128, E], F32)

    qn = const.tile([128, L], F32)      # query natural, tile j at cols 128j
    kn = const.tile([128, L], F32)
    qTd = const.tile([128, L], F32R)    # queryT [d, l] (fp32r for scores)
    kTd = const.tile([128, L], F32)     # keyT   [d, l]
    wqt = const.tile([64, 128], F32)    # W_q^T [e, d]
    wkt = const.tile([64, 128], F32)
    gt = const.tile([128, 128], F32)    # G^T = W_k W_q^T
    nsb = const.tile([128, L], F32R)    # N_t = G @ keyT, tile t at cols 128t
    vag = const.tile([128, 65 * NT], BF16)  # [v_t | ones] per k-tile
    nc.gpsimd.memset(vag[:], 1.0)

    # ---------------- DMA issue ----------------
    q4 = query.rearrange("(c t p) d -> c p t d", t=4, p=128)  # [4,128,4,128]
    k4 = key.rearrange("(c t p) d -> c p t d", t=4, p=128)
    # ACT ring (idle until the exp era): weights then key halves 0,1.
    # SP ring: query halves then key halves 2,3.
    nc.scalar.dma_start(wq[:], wq_d[:])
    nc.scalar.dma_start(wk[:], wk_d[:])
    nc.scalar.dma_start(wv[:], wv_d[:])
    for j in range(2):
        nc.sync.dma_start(
            qn[:, 512 * j:512 * (j + 1)].rearrange("p (t d) -> p t d", d=128), q4[j])
        nc.scalar.dma_start(
            kn[:, 512 * j:512 * (j + 1)].rearrange("p (t d) -> p t d", d=128), k4[j])
    for j in range(2, 4):
        nc.sync.dma_start(
            kn[:, 512 * j:512 * (j + 1)].rearrange("p (t d) -> p t d", d=128), k4[j])
    for j in range(2, 4):
        nc.sync.dma_start(
            qn[:, 512 * j:512 * (j + 1)].rearrange("p (t d) -> p t d", d=128), q4[j])

    # ---------------- PSUM pools (whole kernel) ----------------
    st_pool = ctx.enter_context(tc.tile_pool(name="st", bufs=2, space="PSUM"))
    sc_pool = ctx.enter_context(tc.tile_pool(name="sc", bufs=2, space="PSUM"))
    ac_pool = ctx.enter_context(tc.tile_pool(name="ac", bufs=1, space="PSUM"))
    ex_pool = ctx.enter_context(tc.tile_pool(name="ex", bufs=3))
    ob_pool = ctx.enter_context(tc.tile_pool(name="ob", bufs=2))
    rc_pool = ctx.enter_context(tc.tile_pool(name="rc", bufs=2))

    # PE p-state warm-up: junk transposes with no readers.
    for i in range(NWARM):
        junk = st_pool.tile([128, 128], F32, tag="st", name=f"junk{i}")
        nc.tensor.transpose(junk[:], ident[:], ident[:])

    def copy(eng, dst, src):
        # GPSIMD cannot read PSUM; staging copies go on DVE (or ACT when the
        # exp era hasn't started yet).
        if eng is nc.scalar:
            nc.scalar.copy(dst, src)
        else:
            eng.tensor_copy(dst, src)

    def trans(dst, src_ap, eng=None):
        """PE transpose of [p,f] sbuf block -> [f,p] psum -> copy to dst."""
        np_, nf = src_ap.shape
        p = st_pool.tile([128, 128], F32, tag="st", name="tp")
        nc.tensor.transpose(p[0:nf, 0:np_], src_ap, ident[0:np_, 0:np_])
        copy(eng or nc.vector, dst, p[0:nf, 0:np_])

    def stage_ktile(s):
        """keyT tile s, N_s, vP_s."""
        trans(kTd[:, 128 * s:128 * (s + 1)], kn[:, 128 * s:128 * (s + 1)])
        p = st_pool.tile([128, 128], F32, tag="st", name="np")
        nc.tensor.matmul(p[:], gt[:], kTd[:, 128 * s:128 * (s + 1)],
                         start=True, stop=True)
        nc.vector.tensor_copy(nsb[:, 128 * s:128 * (s + 1)], p[:])
        pv = st_pool.tile([128, E], F32, tag="st", name="vp")
        nc.tensor.matmul(pv[:], kTd[:, 128 * s:128 * (s + 1)], wv[:],
                         start=True, stop=True)
        nc.vector.tensor_copy(vag[:, 65 * s:65 * s + 64], pv[:])

    def stage_qtile(j, eng=None):
        trans(qTd[:, 128 * j:128 * (j + 1)], qn[:, 128 * j:128 * (j + 1)], eng)

    def emit_scores(c, t):
        ps = sc_pool.tile([128, CHUNK], F32, tag="sc", name="ps")
        for j in range(CHUNK // 512):
            qs = slice(CHUNK * c + 512 * j, CHUNK * c + 512 * (j + 1))
            nc.tensor.matmul(
                ps[:, 512 * j:512 * (j + 1)],
                nsb[:, 128 * t:128 * (t + 1)],
                qTd[:, qs],
                start=True, stop=True,
            )
        return ps

    # ---------------- weight prep + phase-1 staging ----------------
    trans(wqt[:], wq[:])
    trans(wkt[:], wk[:])
    pg = st_pool.tile([128, 128], F32, tag="st", name="pg")
    nc.tensor.matmul(pg[:], wkt[:], wqt[:], start=True, stop=True)
    nc.vector.tensor_copy(gt[:], pg[:])

    JIT = True
    if JIT:
        stage_ktile(0)
        for j in range(4):
            stage_qtile(j)
        stage_ktile(1)
        for j in range(4, 8):
            stage_qtile(j)
    else:
        for s in range(NT):
            stage_ktile(s)
        for j in range(NT):
            stage_qtile(j)

    ps = emit_scores(0, 0)

    # ---------------- main loop ----------------
    dbg = ctx.enter_context(tc.tile_pool(name="dbg", bufs=1)) if DEBUG_DUMPS else None
    acc = None
    for c in range(NCHUNK):
        acc = ac_pool.tile([128, CHUNK], F32, tag="ac", name="acc")
        for t in range(NT):
            ex = ex_pool.tile([128, CHUNK], BF16, tag="ex", name="ex")
            nc.scalar.activation(ex[:], ps[:], AF.Exp)
            if DEBUG_DUMPS and (c, t) == (1, 0):
                d_sc = dbg.tile([128, CHUNK], F32, name="d_sc")
                nc.vector.tensor_copy(d_sc[:], ps[:])
                nc.sync.dma_start(dbg_out["sc10"], d_sc[:])
                d_ex = dbg.tile([128, CHUNK], F32, name="d_ex")
                nc.vector.tensor_copy(d_ex[:], ex[:])
                nc.sync.dma_start(dbg_out["ex10"], d_ex[:])

            # PE work independent of ex_t: JIT staging + next scores tile.
            if c == 0 and JIT:
                s = t + 2
                if s < NT:
                    stage_ktile(s)
                if 10 <= t <= 13:
                    stage_qtile(2 * (t - 10) + 8)
                    stage_qtile(2 * (t - 10) + 9)
            nc_, nt_ = (c, t + 1) if t + 1 < NT else (c + 1, 0)
            if nc_ < NCHUNK:
                ps = emit_scores(nc_, nt_)

            # attn@v for tile t (waits on ex_t).  PSUM start=True zeroes the
            # WHOLE bank, so only the first group per bank (j=0 -> bank 0,
            # j=4 -> bank 1) starts; the other groups accumulate onto the
            # zeroed bank with start=False (PE executes in emission order).
            for j in range(NQT):
                nc.tensor.matmul(
                    acc[:, 128 * j:128 * j + 65],
                    ex[:, 128 * j:128 * (j + 1)],
                    vag[:, 65 * t:65 * t + 65],
                    start=(t == 0 and j % 4 == 0), stop=(t == NT - 1),
                    skip_group_check=True,
                )

        if DEBUG_DUMPS:
            d_ac = dbg.tile([128, CHUNK], F32, name="d_ac")
            nc.vector.tensor_copy(d_ac[:], acc[:])
            nc.sync.dma_start(dbg_out[f"acc{c}"], d_ac[:])
        # epilogue for chunk c
        osb = ob_pool.tile([128, 64 * NQT], F32, tag="ob", name="osb")
        for j in range(NQT):
            rec = rc_pool.tile([128, 1], F32, tag="rc", name="rec")
            nc.vector.reciprocal(rec[:], acc[:, 128 * j + 64:128 * j + 65])
            nc.vector.tensor_scalar_mul(
                osb[:, 64 * j:64 * (j + 1)], acc[:, 128 * j:128 * j + 64], rec[:])
        o8 = out.rearrange("(c j p) e -> c p j e", p=128, j=NQT)  # [2,128,8,64]
        nc.sync.dma_start(o8[c], osb[:].rearrange("p (j e) -> p j e", e=64))


def build_nc() -> bass.Bass:
    nc = bacc.Bacc("TRN2", target_bir_lowering=False, debug=False,
                   enable_asserts=False, num_devices=B)
    query = nc.dram_tensor("query", [L, D], F32, kind="ExternalInput").ap()
    key = nc.dram_tensor("key", [L, D], F32, kind="ExternalInput").ap()
    wq = nc.dram_tensor("W_q", [D, E], F32, kind="ExternalInput").ap()
    wk = nc.dram_tensor("W_k", [D, E], F32, kind="ExternalInput").ap()
    wv = nc.dram_tensor("W_v", [D, E], F32, kind="ExternalInput").ap()
    out = nc.dram_tensor("out", [L, E], F32, kind="ExternalOutput").ap()
    dbg_out = None
    if DEBUG_DUMPS:
        dbg_out = {
            n: nc.dram_tensor(n, [128, CHUNK], F32, kind="ExternalOutput").ap()
            for n in ("sc10", "ex10", "acc0", "acc1")
        }
    from contextlib import ExitStack
    with tile.TileContext(nc) as tc:
        with ExitStack() as ctx:
            _build(nc, tc, out, query, key, wq, wk, wv, ctx, dbg_out)
    nc.compile()
    return nc


_NC_CACHE = None


def kernel(**inputs) -> np.ndarray:
    global _NC_CACHE
    if _NC_CACHE is None:
        _NC_CACHE = build_nc()
    nc = _NC_CACHE
    q = np.ascontiguousarray(np.asarray(inputs["query"], dtype=np.float32))
    k = np.ascontiguousarray(np.asarray(inputs["key"], dtype=np.float32))
    wq = np.ascontiguousarray(np.asarray(inputs["W_q"], dtype=np.float32))
    wk = np.ascontiguousarray(np.asarray(inputs["W_k"], dtype=np.float32))
    wv = np.ascontiguousarray(np.asarray(inputs["W_v"], dtype=np.float32))
    in_maps = [
        {"query": q[b], "key": k[b], "W_q": wq, "W_k": wk, "W_v": wv}
        for b in range(B)
    ]
    res = bass_utils.run_bass_kernel_spmd(nc, in_maps, core_ids=list(range(B)))
    return np.stack([r["out"] for r in res.results], axis=0)


# revision 18
# speedup vs baseline: 1.3778x; 1.1350x over previous
"""Cross-attention Bass kernel for Trainium2.

Problem (per batch, data-parallel over 8 batches -> 8 NeuronCores):
    q = query @ W_q          [2048, 64]
    k = key   @ W_k          [2048, 64]
    v = key   @ W_v          [2048, 64]
    scores = q @ k.T         [2048, 2048]
    attn = softmax(scores, axis=-1)
    out = attn @ v           [2048, 64]

Strategy (per core), cost-model driven:
  * Big matmuls run at 1 cycle/row: scores in fp32r (moving free 512),
    attn@v in bf16.  fp32 pays 4 cycles/row on the PE.
  * Both projections fold into the Q side: G = W_q @ W_k^T (one 128x128
    matmul), qG = G^T-contracted queryT (4 fp32r matmuls, free 512).
    scoresT_t = keyT_t^T @ qG, so keyT tiles feed the scores matmul
    directly -- per-k-tile staging is just transpose + vP.
  * attn@v with l_q on PSUM partitions: out[q, 65] accumulated over the
    16 k-tiles, lhsT = exp-scores block (stationary), rhs = [v_t | ones].
    Free dim 65 (vs 1024 in the naive orientation), denominator rides as
    column 64, output lands in natural orientation (no transposes).
  * PSUM discipline: a matmul with start=True zeroes its WHOLE bank, so
    per accumulator bank only the first group starts (j%4==0); the other
    groups ride the zeroed bank with start=False (PE executes in order).
  * ACT (ScalarE) does only the 32 [128,1024] exp instructions
    PSUM->SBUF(bf16) -- the ~33us critical path.  DVE does staging copies
    and the (batched) epilogue; Pool only memsets; SP+ACT issue DMA.
  * Epilogue per chunk: ONE strided reciprocal (8 denominators) + ONE
    broadcast tensor_tensor multiply  -> [128, 512] output staging.
  * PE p-state: junk transposes while DMAs land ramp the clock to 2.4GHz.
  * k/q staging streamed just-in-time through chunk 0 of the main loop.
"""

import numpy as np

import concourse.bass as bass
import concourse.bacc as bacc
import concourse.mybir as mybir
import concourse.tile as tile
from concourse import bass_utils
from concourse.masks import make_identity

F32 = mybir.dt.float32
F32R = mybir.dt.float32r
BF16 = mybir.dt.bfloat16
AF = mybir.ActivationFunctionType
MUL = mybir.AluOpType.mult

B = 8
L = 2048
D = 128
E = 64
NT = L // 128           # 16 k-tiles (and q-tiles)
CHUNK = 1024            # l_q chunk
NCHUNK = L // CHUNK     # 2
NQT = CHUNK // 128      # 8 q-tiles per chunk
NWARM = 10              # junk transposes to ramp the PE clock
DEBUG_DUMPS = False


def _build(nc: bass.Bass, tc: tile.TileContext, out, query, key, wq_d, wk_d, wv_d, ctx,
           dbg_out=None):
    # ---------------- constants ----------------
    const = ctx.enter_context(tc.tile_pool(name="const", bufs=1))
    ident = const.tile([128, 128], F32)
    make_identity(nc, ident[:])

    # Warm the ACT exp table early (pulls the ~1.3us table load into the
    # DMA-wait window).
    warm = const.tile([128, 1], F32)
    nc.vector.memset(warm[:], 0.0)
    nc.scalar.activation(warm[:], warm[:], AF.Exp)

    wq = const.tile([128, E], F32)
    wk = const.tile([128, E], F32)
    wv = const.tile([128, E], F32)
    wvr = const.tile([128, E], F32R)

    qn = const.tile([128, L], F32)      # query natural, tile j at cols 128j
    kn = const.tile([128, L], F32)
    qTd = const.tile([128, L], F32R)    # queryT [d, l]
    kTd = const.tile([128, L], F32R)    # keyT   [d, l] (scores stationary)
    wqt = const.tile([64, 128], F32)    # W_q^T [e, d]
    wkt = const.tile([64, 128], F32)
    gr = const.tile([128, 128], F32R)   # G = W_q W_k^T  [d(q), d'(k)]
    qg = const.tile([128, L], F32R)     # qG [d'(k), l_q]
    vag = const.tile([128, 65 * NT], BF16)  # [v_t | ones] per k-tile

    # ---------------- DMA issue ----------------
    q4 = query.rearrange("(c t p) d -> c p t d", t=4, p=128)  # [4,128,4,128]
    k4 = key.rearrange("(c t p) d -> c p t d", t=4, p=128)

    def ldq(eng, j):
        eng.dma_start(
            qn[:, 512 * j:512 * (j + 1)].rearrange("p (t d) -> p t d", d=128), q4[j])

    def ldk(eng, j):
        eng.dma_start(
            kn[:, 512 * j:512 * (j + 1)].rearrange("p (t d) -> p t d", d=128), k4[j])

    # SP ring: q0 q1 k2 k3 q2 q3 ; ACT ring: wq wk k0 wv k1
    ldq(nc.sync, 0)
    nc.scalar.dma_start(wq[:], wq_d[:])
    nc.scalar.dma_start(wk[:], wk_d[:])
    ldq(nc.sync, 1)
    ldk(nc.scalar, 0)
    nc.scalar.dma_start(wv[:], wv_d[:])
    ldk(nc.sync, 2)
    ldk(nc.scalar, 1)
    ldk(nc.sync, 3)
    ldq(nc.sync, 2)
    ldq(nc.sync, 3)

    # ---------------- PSUM pools (whole kernel) ----------------
    st_pool = ctx.enter_context(tc.tile_pool(name="st", bufs=2, space="PSUM"))
    sc_pool = ctx.enter_context(tc.tile_pool(name="sc", bufs=2, space="PSUM"))
    ac_pool = ctx.enter_context(tc.tile_pool(name="ac", bufs=1, space="PSUM"))
    ex_pool = ctx.enter_context(tc.tile_pool(name="ex", bufs=3))
    ob_pool = ctx.enter_context(tc.tile_pool(name="ob", bufs=2))
    rc_pool = ctx.enter_context(tc.tile_pool(name="rc", bufs=2))

    # PE p-state warm-up: junk transposes with no readers.
    for i in range(NWARM):
        junk = st_pool.tile([128, 128], F32, tag="st", name=f"junk{i}")
        nc.tensor.transpose(junk[:], ident[:], ident[:])

    # vag ones (emitted after identity so Pool unblocks the PE warm-up first)
    nc.gpsimd.memset(vag[:], 1.0)

    def copy(eng, dst, src):
        # GPSIMD cannot read PSUM; staging copies go on DVE (or ACT when the
        # exp era hasn't started yet).
        if eng is nc.scalar:
            nc.scalar.copy(dst, src)
        else:
            eng.tensor_copy(dst, src)

    def trans(dst, src_ap, eng=None):
        """PE transpose of [p,f] sbuf block -> [f,p] psum -> copy to dst."""
        np_, nf = src_ap.shape
        p = st_pool.tile([128, 128], F32, tag="st", name="tp")
        nc.tensor.transpose(p[0:nf, 0:np_], src_ap, ident[0:np_, 0:np_])
        copy(eng or nc.vector, dst, p[0:nf, 0:np_])

    def stage_ktile(s, eng=None):
        """keyT tile s and vP_s."""
        trans(kTd[:, 128 * s:128 * (s + 1)], kn[:, 128 * s:128 * (s + 1)], eng)
        pv = st_pool.tile([128, E], F32, tag="st", name="vp")
        nc.tensor.matmul(pv[:], kTd[:, 128 * s:128 * (s + 1)], wvr[:],
                         start=True, stop=True)
        copy(eng or nc.vector, vag[:, 65 * s:65 * s + 64], pv[:])

    def stage_qtile(j, eng=None):
        trans(qTd[:, 128 * j:128 * (j + 1)], qn[:, 128 * j:128 * (j + 1)], eng)

    def stage_qg(b, eng=None):
        """qG block b: [d', 512] = G^T-contraction with qTd cols 512b.."""
        p = st_pool.tile([128, 512], F32, tag="st", name="qg")
        nc.tensor.matmul(p[:], gr[:], qTd[:, 512 * b:512 * (b + 1)],
                         start=True, stop=True)
        copy(eng or nc.vector, qg[:, 512 * b:512 * (b + 1)], p[:])

    def emit_scores(c, t):
        ps = sc_pool.tile([128, CHUNK], F32, tag="sc", name="ps")
        for j in range(CHUNK // 512):
            qs = slice(CHUNK * c + 512 * j, CHUNK * c + 512 * (j + 1))
            nc.tensor.matmul(
                ps[:, 512 * j:512 * (j + 1)],
                kTd[:, 128 * t:128 * (t + 1)],
                qg[:, qs],
                start=True, stop=True,
            )
        return ps

    # ---------------- weight prep + phase-1 staging ----------------
    trans(wqt[:], wq[:])
    trans(wkt[:], wk[:])
    pg = st_pool.tile([128, 128], F32, tag="st", name="pg")
    nc.tensor.matmul(pg[:], wqt[:], wkt[:], start=True, stop=True)
    nc.vector.tensor_copy(gr[:], pg[:])
    nc.vector.tensor_copy(wvr[:], wv[:])

    for j in range(4):
        stage_qtile(j, nc.scalar if j % 2 else None)
    stage_qg(0)
    stage_ktile(0, nc.scalar)
    for j in range(4, 8):
        stage_qtile(j, nc.scalar if j % 2 else None)
    stage_qg(1, nc.scalar)

    ps = emit_scores(0, 0)
    stage_ktile(1)

    # ---------------- main loop ----------------
    dbg = ctx.enter_context(tc.tile_pool(name="dbg", bufs=1)) if DEBUG_DUMPS else None
    o8 = out.rearrange("(c j p) e -> c p j e", p=128, j=NQT)  # [2,128,8,64]
    for c in range(NCHUNK):
        acc = ac_pool.tile([128, CHUNK], F32, tag="ac", name="acc")
        for t in range(NT):
            ex = ex_pool.tile([128, CHUNK], BF16, tag="ex", name="ex")
            nc.scalar.activation(ex[:], ps[:], AF.Exp)
            if DEBUG_DUMPS and (c, t) == (1, 0):
                d_sc = dbg.tile([128, CHUNK], F32, name="d_sc")
                nc.vector.tensor_copy(d_sc[:], ps[:])
                nc.sync.dma_start(dbg_out["sc10"], d_sc[:])
                d_ex = dbg.tile([128, CHUNK], F32, name="d_ex")
                nc.vector.tensor_copy(d_ex[:], ex[:])
                nc.sync.dma_start(dbg_out["ex10"], d_ex[:])

            # PE work independent of ex_t: JIT staging + next scores tile.
            if c == 0:
                s = t + 2
                if s < NT:
                    stage_ktile(s)
                if 6 <= t <= 13:
                    stage_qtile(t + 2)        # q-tiles 8..15
                if t == 11:
                    stage_qg(2)
                if t == 14:
                    stage_qg(3)
            nc_, nt_ = (c, t + 1) if t + 1 < NT else (c + 1, 0)
            if nc_ < NCHUNK:
                ps = emit_scores(nc_, nt_)

            # attn@v for tile t (waits on ex_t).  Only the first group per
            # accumulator bank starts (start=True zeroes the whole bank).
            for j in range(NQT):
                nc.tensor.matmul(
                    acc[:, 128 * j:128 * j + 65],
                    ex[:, 128 * j:128 * (j + 1)],
                    vag[:, 65 * t:65 * t + 65],
                    start=(t == 0 and j % 4 == 0), stop=(t == NT - 1),
                    skip_group_check=True,
                )

        if DEBUG_DUMPS:
            d_ac = dbg.tile([128, CHUNK], F32, name="d_ac")
            nc.vector.tensor_copy(d_ac[:], acc[:])
            nc.sync.dma_start(dbg_out[f"acc{c}"], d_ac[:])

        # epilogue for chunk c: batched reciprocal + broadcast multiply
        acr = acc[:].rearrange("p (j f) -> p j f", f=128)      # [128, 8, 128]
        rec = rc_pool.tile([128, NQT], F32, tag="rc", name="rec")
        nc.vector.reciprocal(rec[:], acr[:, :, 64:65])
        osb = ob_pool.tile([128, 64 * NQT], F32, tag="ob", name="osb")
        nc.vector.tensor_tensor(
            out=osb[:].rearrange("p (j e) -> p j e", e=64),
            in0=acr[:, :, 0:64],
            in1=rec[:].rearrange("p (j e) -> p j e", e=1).broadcast_to([128, NQT, 64]),
            op=MUL,
        )
        if c < NCHUNK - 1:
            nc.sync.dma_start(o8[c], osb[:].rearrange("p (j e) -> p j e", e=64))
        else:
            # final chunk: split across both rings (ACT is done with exps)
            nc.sync.dma_start(
                o8[c][:, 0:4], osb[:, 0:256].rearrange("p (j e) -> p j e", e=64))
            nc.scalar.dma_start(
                o8[c][:, 4:8], osb[:, 256:512].rearrange("p (j e) -> p j e", e=64))


def build_nc() -> bass.Bass:
    nc = bacc.Bacc("TRN2", target_bir_lowering=False, debug=False,
                   enable_asserts=False, num_devices=B)
    query = nc.dram_tensor("query", [L, D], F32, kind="ExternalInput").ap()
    key = nc.dram_tensor("key", [L, D], F32, kind="ExternalInput").ap()
    wq = nc.dram_tensor("W_q", [D, E], F32, kind="ExternalInput").ap()
    wk = nc.dram_tensor("W_k", [D, E], F32, kind="ExternalInput").ap()
    wv = nc.dram_tensor("W_v", [D, E], F32, kind="ExternalInput").ap()
    out = nc.dram_tensor("out", [L, E], F32, kind="ExternalOutput").ap()
    dbg_out = None
    if DEBUG_DUMPS:
        dbg_out = {
            n: nc.dram_tensor(n, [128, CHUNK], F32, kind="ExternalOutput").ap()
            for n in ("sc10", "ex10", "acc0", "acc1")
        }
    from contextlib import ExitStack
    with tile.TileContext(nc) as tc:
        with ExitStack() as ctx:
            _build(nc, tc, out, query, key, wq, wk, wv, ctx, dbg_out)
    nc.compile()
    return nc


_NC_CACHE = None


def kernel(**inputs) -> np.ndarray:
    global _NC_CACHE
    if _NC_CACHE is None:
        _NC_CACHE = build_nc()
    nc = _NC_CACHE
    q = np.ascontiguousarray(np.asarray(inputs["query"], dtype=np.float32))
    k = np.ascontiguousarray(np.asarray(inputs["key"], dtype=np.float32))
    wq = np.ascontiguousarray(np.asarray(inputs["W_q"], dtype=np.float32))
    wk = np.ascontiguousarray(np.asarray(inputs["W_k"], dtype=np.float32))
    wv = np.ascontiguousarray(np.asarray(inputs["W_v"], dtype=np.float32))
    in_maps = [
        {"query": q[b], "key": k[b], "W_q": wq, "W_k": wk, "W_v": wv}
        for b in range(B)
    ]
    res = bass_utils.run_bass_kernel_spmd(nc, in_maps, core_ids=list(range(B)))
    return np.stack([r["out"] for r in res.results], axis=0)
